# revision 26
# baseline (speedup 1.0000x reference)
"""Trainium2 Bass kernel for nn_Decoder (attention LSTM decoder, LAS-style).

Sharding: data-parallel over batch B=128 across 8 NeuronCores (16 batch
elements per core, length-sorted snake assignment for load balance).

v3 structure (vs v2):
- Doubled-state LSTM: store H1=2*h1, H2=2*h2, C=2*c. sigmoid(x)*y is
  computed as 0.5*(tanh(x/2)+1)*y via scalar_tensor_tensor ops; the 0.5
  factors fold into host-prescaled weights (Whh1, Wih2, Whh2, K, Wlin-h2
  chunk all *0.5). Removes the sigmoid-affine DVE op from the chain.
- PSUM bias/mask adds folded into PE identity-matmuls (bf16 hi+lo pairs
  for g1e and b2 to keep fp32-level exactness; single bf16 for the
  attention mask). These matmuls open each PSUM accumulation group and
  are issued early, shortening the cross-engine serial chain.
- gates2 bias+h2-part matmuls pre-issued right after the previous
  step's pointwise (ps_g2 double-buffered); gates1 h1-part pre-issued
  as before.
- UNROLL=16 (half the loop back-edges / IRAM refetch stalls).
"""

import sys

sys.path.insert(0, "/opt/trn_rl_repo")

import numpy as np
import ml_dtypes

import concourse.bass as bass
import concourse.mybir as mybir
import concourse.tile as tile
from concourse.bass_utils import run_bass_kernel_spmd
from concourse.vector_clock import ScopedClock

bf16 = ml_dtypes.bfloat16
FP32 = mybir.dt.float32
BF16 = mybir.dt.bfloat16
FP8 = mybir.dt.float8e3
f8 = ml_dtypes.float8_e3m4

# Problem constants (hardcoded per harness contract)
VOCAB = 1000
HID = 256
VAL = 128
KEY = 128
B = 128
T_ENC = 2048
T_DEC = 256
H1 = 512  # lstm1 hidden
N_CORES = 8
B_LOC = B // N_CORES  # 16
UNROLL = 16  # steps per For_i block
NVT = 8  # vocab tiles (7*128 + 104)
NTMAX = 16  # rect attention tiles per slot

_tanh = mybir.ActivationFunctionType.Tanh
_exp = mybir.ActivationFunctionType.Exp
_mult = mybir.AluOpType.mult
_add = mybir.AluOpType.add
_ax = mybir.AxisListType.X


def _patch_tile_drain():
    """Walrus in this env rejects >1 sync wait on the kernel-tail Drain.
    Split the aggregated waits onto individual NoOps before the drain."""

    def _patched(self, tick_clock, wait_clock):
        nop1 = self.nc.sync.nop()
        wait_clock.add_sem_waits(nop1.ins, ScopedClock({None: tick_clock.global_clock}))
        si = nop1.ins.sync_info
        waits = list(si.on_wait) if si and si.on_wait else []
        if len(waits) > 1:
            si.on_wait = waits[:1]
            for w in waits[1:]:
                n = self.nc.sync.nop()
                nsi = n.ins.sync_info
                if nsi is None:
                    n.ins.sync_info = mybir.SyncInfo(on_wait=[w], on_update=[])
                else:
                    nsi.on_wait = list(nsi.on_wait or []) + [w]
        self.nc.sync.drain()
        self.nc.all_engine_barrier()
        popped = self.nc._tile_sem_poison_stack.pop()
        assert popped is self._sem_poison
        self.nc.clear_and_free_semaphores(list(self.sems.allocated().values()))
        self.nc.all_engine_barrier()

    tile.TileContext._drain_and_barrier = _patched


_patch_tile_drain()

TRACE = False
LAST_EXEC_NS = None
SPLIT_WAITS = True


def _split_drain_waits(nc):
    """Walrus in this env rejects >1 sync wait per instruction. Split the
    waits of any multi-wait instruction onto single-wait NoOps that execute
    just before it on the same engine."""
    n = 0
    for f in nc.m.functions:
        for bb in f.blocks:
            newlist = []
            for inst in bb.instructions:
                si = getattr(inst, "sync_info", None)
                eng = getattr(inst, "engine", None)
                if (si and si.on_wait and len(si.on_wait) > 1
                        and eng is not None
                        and eng != mybir.EngineType.Unassigned):
                    waits = list(si.on_wait)
                    si.on_wait = waits[-1:]
                    for k, w in enumerate(waits[:-1]):
                        n += 1
                        newlist.append(mybir.InstNoOp(
                            name=f"{inst.name}_dw{k}", engine=eng,
                            sync_info=mybir.SyncInfo(on_wait=[w], on_update=[]),
                            bass_nofuse=True))
                newlist.append(inst)
            bb.instructions[:] = newlist
    return n


def build_program(NT, t_dec=T_DEC, unroll=UNROLL):
    """NT: list of 16 per-slot tile counts (ceil(max len in slot group /128)).
    Same program runs SPMD on all 8 cores."""
    TOT = int(sum(NT))
    off = np.concatenate([[0], np.cumsum(NT)]).astype(int)  # tile col offsets

    nc = bass.Bass("TRN2", target_bir_lowering=False, debug=False,
                   enable_asserts=False, num_devices=N_CORES)

    # ---- DRAM I/O ----
    K_d = nc.declare_dram_parameter("K", [128, TOT * 128], BF16, isOutput=False)
    V_d = nc.declare_dram_parameter("V", [128, TOT * 128], BF16, isOutput=False)
    W1_d = nc.declare_dram_parameter("W1T", [128, 5 * 2048], BF16, isOutput=False)
    W2_d = nc.declare_dram_parameter("W2T", [128, 5 * 512], BF16, isOutput=False)
    WL_d = nc.declare_dram_parameter("WLT", [128, 2 * VOCAB], BF16, isOutput=False)
    ID_d = nc.declare_dram_parameter("ID", [128, 128], BF16, isOutput=False)
    MSK_d = nc.declare_dram_parameter("MSK", [128, 2 * 128], BF16, isOutput=False)
    B2_d = nc.declare_dram_parameter("B2", [128, 4 * B_LOC], BF16, isOutput=False)
    BL_d = nc.declare_dram_parameter("BL", [128, NVT * B_LOC], FP32, isOutput=False)
    n_blk = t_dec // unroll
    # g1 embedding contribution (bf16): [p, blk, t, m, j]
    G1E_d = nc.declare_dram_parameter("G1E", [128, n_blk, unroll, 16, B_LOC],
                                      BF16, isOutput=False)
    OUT_d = nc.declare_dram_parameter("OUT", [n_blk, 128, unroll, NVT, B_LOC],
                                      FP32, isOutput=True)

    from contextlib import ExitStack
    with tile.TileContext(nc) as tc, ExitStack() as ctx:
        res = ctx.enter_context(tc.tile_pool(name="res", bufs=1))
        state = ctx.enter_context(tc.tile_pool(name="state", bufs=1))
        work = ctx.enter_context(tc.tile_pool(name="work", bufs=2))
        expp = ctx.enter_context(tc.tile_pool(name="expp", bufs=2))
        embp = ctx.enter_context(tc.tile_pool(name="embp", bufs=2))
        stgp = ctx.enter_context(tc.tile_pool(name="stgp", bufs=2))
        # PSUM budget is 8 banks: g1 x2, g2 x2, e x2, cx 1, wl 1
        ps_g1 = ctx.enter_context(tc.tile_pool(name="ps_g1", bufs=2, space="PSUM"))
        ps_g2 = ctx.enter_context(tc.tile_pool(name="ps_g2", bufs=2, space="PSUM"))
        ps_e = ctx.enter_context(tc.tile_pool(name="ps_e", bufs=2, space="PSUM"))
        ps_cx = ctx.enter_context(tc.tile_pool(name="ps_cx", bufs=1, space="PSUM"))
        ps_wl = ctx.enter_context(tc.tile_pool(name="ps_wl", bufs=1, space="PSUM"))

        # ---- resident tiles ----
        K_sb = res.tile([128, TOT * 128], BF16)
        V_sb = res.tile([128, TOT * 128], BF16)
        W1_sb = res.tile([128, 5, 2048], BF16)   # chunks: ctx, h1 x4
        W2_sb = res.tile([128, 5, 512], BF16)    # chunks: h1 x4, h2
        WL_sb = res.tile([128, 2, VOCAB], BF16)  # chunks: h2, ctx
        ID_sb = res.tile([128, 128], BF16)
        MSK_sb = res.tile([128, 2, 128], BF16)   # per half: (jj, tt) flat
        B2_sb = res.tile([128, 4 * B_LOC], BF16)
        BL_sb = res.tile([128, NVT, B_LOC], FP32)
        ONES_sb = res.tile([128, 128], FP32)

        nc.sync.dma_start(out=K_sb, in_=K_d[:, :])
        nc.sync.dma_start(out=V_sb, in_=V_d[:, :])
        nc.sync.dma_start(out=W1_sb, in_=W1_d[:, :].rearrange("p (c m) -> p c m", c=5))
        nc.sync.dma_start(out=W2_sb, in_=W2_d[:, :].rearrange("p (c m) -> p c m", c=5))
        nc.sync.dma_start(out=WL_sb, in_=WL_d[:, :].rearrange("p (c m) -> p c m", c=2))
        nc.sync.dma_start(out=ID_sb, in_=ID_d[:, :])
        nc.sync.dma_start(out=MSK_sb, in_=MSK_d[:, :].rearrange("p (h m) -> p h m", h=2))
        nc.sync.dma_start(out=B2_sb, in_=B2_d[:, :])
        nc.sync.dma_start(out=BL_sb, in_=BL_d[:, :].rearrange("p (m j) -> p m j", m=NVT))
        nc.vector.memset(ONES_sb, 1.0)

        # ---- recurrent state (H1 = 2*h1, H2 = 2*h2, C = 2*c) ----
        h1_sb = state.tile([128, 4, B_LOC], BF16)   # [H1 chunk part, chunk, slot]
        c1_sb = state.tile([128, 4, B_LOC], FP32)
        h2_sb = state.tile([128, B_LOC], BF16)      # [KEY part, slot]
        c2_sb = state.tile([128, B_LOC], FP32)
        ctx_sb = state.tile([128, B_LOC], BF16)     # [VAL part, slot]
        nc.vector.memset(h1_sb, 0.0)
        nc.vector.memset(c1_sb, 0.0)
        nc.vector.memset(h2_sb, 0.0)
        nc.vector.memset(c2_sb, 0.0)
        nc.vector.memset(ctx_sb, 0.0)

        def emit_g1_open(g1, g1e, j):
            # opens the g1 accumulation group with the embedding contribution
            nc.tensor.matmul(g1[:, :, :], ID_sb, g1e[:, j, :, :],
                             start=True, stop=False)

        def emit_g1_h1part(g1):
            # gates1 += Whh1' @ H1 (4 chunks)
            for m in range(16):
                for c in range(4):
                    nc.tensor.matmul(
                        g1[:, m, :], W1_sb[:, 1 + c, m * 128:(m + 1) * 128],
                        h1_sb[:, c, :], start=False, stop=False)

        def emit_g2_open(g2):
            # b2 identity-matmul + Whh2' @ H2 part
            nc.tensor.matmul(g2[:, :, :], ID_sb, B2_sb[:, :],
                             start=True, stop=False)
            for m in range(4):
                nc.tensor.matmul(
                    g2[:, m, :], W2_sb[:, 4, m * 128:(m + 1) * 128],
                    h2_sb[:, :], start=False, stop=False)

        def emit_proj(wl):
            rhsl = [h2_sb[:, :], ctx_sb[:, :]]
            for vt in range(NVT):
                mdim = 128 if vt < 7 else VOCAB - 7 * 128
                for c in range(2):
                    nc.tensor.matmul(
                        wl[0:mdim, vt, :], WL_sb[:, c, vt * 128:vt * 128 + mdim],
                        rhsl[c], start=(c == 0), stop=(c == 1))

        def emit_masks():
            # pre-seed both energy PSUM tiles with the attention mask
            # (row 8 of each tile is scratch: half B's holds the softmax
            # denominators so the reciprocal can overlap the ctx matmuls)
            pair = []
            for half in range(2):
                ep = ps_e.tile([128, 9, NTMAX], FP32, tag="ep")
                nc.tensor.matmul(ep[:, 0:8, :], ID_sb, MSK_sb[:, half, :],
                                 start=True, stop=False)
                pair.append(ep)
            return pair

        hint = (mybir.EngineType.PE, mybir.EngineType.DVE,
                mybir.EngineType.Activation, mybir.EngineType.SP)
        with tc.For_i(0, n_blk, 1, hint_engines=hint,
                      staggered_reset=True) as iv:
            g1e = embp.tile([128, unroll, 16, B_LOC], BF16, tag="g1e")
            nc.sync.dma_start(
                out=g1e,
                in_=G1E_d[:, bass.ds(iv, 1), :, :, :].rearrange(
                    "p 1 t m j -> p t m j"))
            stg = stgp.tile([128, unroll, NVT, B_LOC], FP32, tag="stg")

            g1_cur = None
            g2_cur = None
            wl_prev = None
            ep_pair = None
            for j in range(unroll):
                # ---- A: finish gates1 of step j (ctx part) ----
                if g1_cur is None:  # j == 0: nothing pre-emitted
                    g1_cur = ps_g1.tile([128, 16, B_LOC], FP32, tag="g1")
                    emit_g1_open(g1_cur, g1e, j)
                    emit_g1_h1part(g1_cur)
                for m in range(16):
                    nc.tensor.matmul(
                        g1_cur[:, m, :], W1_sb[:, 0, m * 128:(m + 1) * 128],
                        ctx_sb[:, :], start=False, stop=True)

                # ---- B: LSTM1 pointwise -> H1, C1 (doubled states) ----
                # g-gate rows are host-predoubled, so tanh(g/2) covers all 16
                t1 = work.tile([128, 16, B_LOC], FP32, tag="t1")
                nc.scalar.activation(t1[:, :, :], g1_cur[:, :, :], _tanh,
                                     scale=0.5)
                u1 = work.tile([128, 4, B_LOC], FP32, tag="u1")
                nc.vector.scalar_tensor_tensor(
                    u1[:, :, :], t1[:, 4:8, :], 1.0, c1_sb[:, :, :], _add, _mult)
                v1 = work.tile([128, 4, B_LOC], FP32, tag="v1")
                nc.vector.scalar_tensor_tensor(
                    v1[:, :, :], t1[:, 0:4, :], 1.0, t1[:, 12:16, :], _add, _mult)
                nc.vector.scalar_tensor_tensor(
                    c1_sb[:, :, :], u1[:, :, :], 0.5, v1[:, :, :], _mult, _add)
                tc1 = work.tile([128, 4, B_LOC], FP32, tag="tc1")
                nc.scalar.activation(tc1[:, :, :], c1_sb[:, :, :], _tanh, scale=0.5)
                nc.vector.scalar_tensor_tensor(
                    h1_sb[:, :, :], t1[:, 8:12, :], 1.0, tc1[:, :, :], _add, _mult)

                # ---- C: output projection of step j-1 + this step's energy
                #      mask seeds (both fill the pw1 window)
                if j >= 1:
                    wl_prev = ps_wl.tile([128, NVT, B_LOC], FP32, tag="wl")
                    emit_proj(wl_prev)
                if ep_pair is None:
                    ep_pair = emit_masks()

                # ---- D: gates2 h1-part matmuls (b2 + h2 part pre-issued) ----
                if g2_cur is None:  # j == 0
                    g2_cur = ps_g2.tile([128, 4, B_LOC], FP32, tag="g2")
                    emit_g2_open(g2_cur)
                for m in range(4):
                    for c in range(4):
                        nc.tensor.matmul(
                            g2_cur[:, m, :], W2_sb[:, c, m * 128:(m + 1) * 128],
                            h1_sb[:, c, :], start=False, stop=(c == 3))

                # ---- E: LSTM2 pointwise -> H2, C2 ----
                t2 = work.tile([128, 4, B_LOC], FP32, tag="t2")
                nc.scalar.activation(t2[:, :, :], g2_cur[:, :, :], _tanh,
                                     scale=0.5)
                u2 = work.tile([128, B_LOC], FP32, tag="u2")
                nc.vector.scalar_tensor_tensor(
                    u2[:, :], t2[:, 1, :], 1.0, c2_sb[:, :], _add, _mult)
                v2 = work.tile([128, B_LOC], FP32, tag="v2")
                nc.vector.scalar_tensor_tensor(
                    v2[:, :], t2[:, 0, :], 1.0, t2[:, 3, :], _add, _mult)
                nc.vector.scalar_tensor_tensor(
                    c2_sb[:, :], u2[:, :], 0.5, v2[:, :], _mult, _add)
                tc2 = work.tile([128, B_LOC], FP32, tag="tc2")
                nc.scalar.activation(tc2[:, :], c2_sb[:, :], _tanh, scale=0.5)
                nc.vector.scalar_tensor_tensor(
                    h2_sb[:, :], t2[:, 2, :], 1.0, tc2[:, :], _add, _mult)

                # ---- D2: open next step's gates1 (fills pw2/attention window)
                if j < unroll - 1:
                    g1_cur = ps_g1.tile([128, 16, B_LOC], FP32, tag="g1")
                    emit_g1_open(g1_cur, g1e, j + 1)
                    emit_g1_h1part(g1_cur)
                else:
                    g1_cur = None

                # ---- F/G/H: attention ----
                # PE order: energy A, energy B, ctx A, Z ones-MM, ctx B.
                # exp/reduce of each half hide under the next PE burst; the
                # reciprocal overlaps ctx B.
                if ep_pair is None:  # j == 0 of a block
                    ep_pair = emit_masks()
                att = expp.tile([128, B_LOC, NTMAX], BF16, tag="att")
                RS = work.tile([128, B_LOC], FP32, tag="RS")
                cxs = ps_cx.tile([128, B_LOC], FP32, tag="cxs")
                for half in range(2):
                    h0 = half * 8
                    ep = ep_pair[half]
                    for jj in range(8):
                        j2 = h0 + jj
                        for tt in range(int(NT[j2])):
                            col = (int(off[j2]) + tt) * 128
                            nc.tensor.matmul(
                                ep[:, jj, tt:tt + 1], K_sb[:, col:col + 128],
                                h2_sb[:, j2:j2 + 1], start=False, stop=True,
                                skip_group_check=True)
                    nc.scalar.activation(att[:, h0:h0 + 8, :], ep[:, 0:8, :], _exp)
                    nc.vector.reduce_sum(RS[:, h0:h0 + 8], att[:, h0:h0 + 8, :],
                                         axis=_ax)
                for half in range(2):
                    h0 = half * 8
                    for jj in range(8):
                        j2 = h0 + jj
                        ntj = int(NT[j2])
                        for tt in range(ntj):
                            col = (int(off[j2]) + tt) * 128
                            nc.tensor.matmul(
                                cxs[:, j2:j2 + 1], V_sb[:, col:col + 128],
                                att[:, j2, tt:tt + 1],
                                start=(tt == 0), stop=(tt == ntj - 1))
                    if half == 0:
                        # Z into ep half-B scratch row; reciprocal then
                        # overlaps the half-B ctx matmuls
                        nc.tensor.matmul(ep_pair[1][:, 8, :], ONES_sb[:, :],
                                         RS[:, :], start=True, stop=True,
                                         skip_group_check=True)
                        rS = work.tile([128, B_LOC], FP32, tag="rS")
                        nc.vector.reciprocal(rS[:, :], ep_pair[1][:, 8, :])
                nc.vector.tensor_mul(ctx_sb[:, :], cxs[:, :], rS[:, :])

                # ---- D3: open next step's gates2 (b2 + h2 part) and
                #      pre-seed next step's energy masks (fills the softmax
                #      tail window) ----
                if j < unroll - 1:
                    g2_cur = ps_g2.tile([128, 4, B_LOC], FP32, tag="g2")
                    emit_g2_open(g2_cur)
                else:
                    g2_cur = None
                ep_pair = None

                # ---- L: logits bias add for step j-1 ----
                if j >= 1:
                    nc.vector.tensor_add(stg[:, j - 1, :, :], wl_prev[:, :, :],
                                         BL_sb[:, :, :])

                # staggered-reset stage boundaries (4 stages x unroll/4 steps)
                if j % 4 == 3 and j != unroll - 1:
                    tc.stage_boundary()

            # ---- block tail: projection of last step + DMA out ----
            wl_last = ps_wl.tile([128, NVT, B_LOC], FP32, tag="wl")
            emit_proj(wl_last)
            nc.vector.tensor_add(stg[:, unroll - 1, :, :], wl_last[:, :, :],
                                 BL_sb[:, :, :])
            nc.sync.dma_start(
                out=OUT_d[bass.ds(iv, 1), :, :, :, :].rearrange(
                    "1 p t m j -> p t m j"),
                in_=stg)

    if SPLIT_WAITS:
        _split_drain_waits(nc)
    return nc


def _prep_core_arrays(core, slots, NT, off, keys, values, lens,
                      ge_all, W1T, W2T, WLT, b2bc, blbc, id_a):
    TOT = int(sum(NT))
    K_a = np.zeros((128, TOT * 128), dtype=bf16)
    V_a = np.zeros((128, TOT * 128), dtype=bf16)
    M_a = np.full((128, B_LOC, NTMAX), -1e9, dtype=np.float32)
    for j, gb in enumerate(slots):
        for tt in range(int(NT[j])):
            col = (int(off[j]) + tt) * 128
            t0 = tt * 128
            # K prescaled by 0.5 (H2 is stored doubled)
            K_a[:, col:col + 128] = (0.5 * keys[t0:t0 + 128, gb, :].T).astype(bf16)
            V_a[:, col:col + 128] = values[t0:t0 + 128, gb, :].astype(bf16)
            tpos = np.arange(t0, t0 + 128)
            M_a[:, j, tt] = np.where(tpos < int(lens[gb]), 0.0, -1e9)
    # mask rect layout per half: [p, half, (jj, tt)]
    M_h = np.stack([M_a[:, 0:8, :].reshape(128, 128),
                    M_a[:, 8:16, :].reshape(128, 128)], axis=1)
    # g1emb: [p, blk, t, m, j] = ge_all[batch, blk*unroll+t, m*128+p]
    ge = ge_all[slots]                               # (16, T_dec, 2048)
    ge = ge.reshape(B_LOC, -1, UNROLL, 16, 128)      # (j, b, t, m, p)
    g1e = np.ascontiguousarray(
        ge.transpose(4, 1, 2, 3, 0)).astype(bf16)    # (p, blk, t, m, j)
    return {
        "K": K_a, "V": V_a, "W1T": W1T, "W2T": W2T, "WLT": WLT, "ID": id_a,
        "MSK": M_h.astype(bf16).reshape(128, 2 * 128),
        "B2": b2bc, "BL": blbc, "G1E": g1e,
    }


def kernel(keys, values, lens, text, emb_table,
           Wih1, Whh1, bih1, bhh1, Wih2, Whh2, bih2, bhh2, Wlin, blin):
    keys = np.asarray(keys, np.float32)
    values = np.asarray(values, np.float32)
    lens_i = np.asarray(lens).astype(np.int64)
    text_i = np.asarray(text).astype(np.int64)

    # batch assignment: sort desc by len, snake over cores within groups of 8
    order = np.argsort(-lens_i, kind="stable")
    NT = np.zeros(B_LOC, dtype=int)
    core_slots = [[0] * B_LOC for _ in range(N_CORES)]
    for j in range(B_LOC):
        grp = order[j * N_CORES:(j + 1) * N_CORES]
        NT[j] = max(1, int(np.ceil(int(lens_i[grp[0]]) / 128)))
        for c in range(N_CORES):
            core_slots[c][j] = int(grp[c] if j % 2 == 0 else grp[N_CORES - 1 - c])
    off = np.concatenate([[0], np.cumsum(NT)]).astype(int)

    # gate reorder [i, f, o, g]
    perm1 = np.concatenate([np.arange(0, 512), np.arange(512, 1024),
                            np.arange(1536, 2048), np.arange(1024, 1536)])
    perm2 = np.concatenate([np.arange(0, 128), np.arange(128, 256),
                            np.arange(384, 512), np.arange(256, 384)])

    Wih1_f = np.asarray(Wih1, np.float32)[perm1]        # (2048, 384)
    Whh1_f = np.asarray(Whh1, np.float32)[perm1]        # (2048, 512)
    Wih2_f = np.asarray(Wih2, np.float32)[perm2]        # (512, 512)
    Whh2_f = np.asarray(Whh2, np.float32)[perm2]        # (512, 128)
    b1p = (np.asarray(bih1, np.float32) + np.asarray(bhh1, np.float32))[perm1]
    b2p = (np.asarray(bih2, np.float32) + np.asarray(bhh2, np.float32))[perm2]

    # W1 device chunks: [ctx (Wih1 cols 256:384), h1 x4 (0.5*Whh1 — H1 doubled)]
    # g-gate rows (1536:2048) doubled so one tanh(x/2) covers all gates
    W1x = np.concatenate([Wih1_f[:, 256:384], 0.5 * Whh1_f], axis=1)  # (2048, 640)
    W1x[1536:2048, :] *= 2.0
    W1T = np.ascontiguousarray(
        W1x.T.reshape(5, 128, 2048).transpose(1, 0, 2).reshape(128, 5 * 2048)
    ).astype(bf16)
    # W2 device chunks: [h1 x4 (0.5*Wih2), h2 (0.5*Whh2)] — both states doubled
    W2x = np.concatenate([0.5 * Wih2_f, 0.5 * Whh2_f], axis=1)        # (512, 640)
    W2x[384:512, :] *= 2.0
    W2T = np.ascontiguousarray(
        W2x.T.reshape(5, 128, 512).transpose(1, 0, 2).reshape(128, 5 * 512)
    ).astype(bf16)
    # Wlin chunks: h2 chunk * 0.5 (H2 doubled), ctx chunk unchanged
    WLTf = np.ascontiguousarray(np.asarray(Wlin, np.float32).T)  # (256, 1000)
    WLTf[0:128, :] *= 0.5
    WLT = np.ascontiguousarray(
        WLTf.astype(bf16).reshape(2, 128, VOCAB).transpose(1, 0, 2)
        .reshape(128, 2 * VOCAB))

    # b2 broadcast [p, m, j] bf16, g-gate chunk doubled
    b2f = np.repeat(b2p.reshape(4, 128, 1), B_LOC, axis=2).transpose(1, 0, 2) \
        .astype(np.float32)                          # (128, 4, 16)
    b2f[:, 3, :] *= 2.0
    b2bc = np.ascontiguousarray(b2f.reshape(128, 4 * B_LOC).astype(bf16))
    blv = np.asarray(blin, np.float32)
    blbc = np.zeros((128, NVT, B_LOC), np.float32)
    for vt in range(NVT):
        n = 128 if vt < 7 else VOCAB - 7 * 128
        blbc[0:n, vt, :] = blv[vt * 128:vt * 128 + n, None]
    blbc = np.ascontiguousarray(blbc.reshape(128, NVT * B_LOC))

    id_a = np.eye(128, dtype=bf16)

    # host precompute: g1 embedding part = Wih1[:, :256] @ emb + b1
    # (g-gate rows doubled to match the device-side single-tanh convention)
    emb_np = np.asarray(emb_table, np.float32)[text_i]  # (B, T_dec, 256)
    ge_all = (emb_np.reshape(-1, HID) @ Wih1_f[:, :HID].T + b1p) \
        .reshape(B, T_DEC, 4 * H1)                      # (B, T_dec, 2048)
    ge_all[:, :, 1536:2048] *= 2.0

    nc = build_program(list(NT))
    in_maps = [
        _prep_core_arrays(c, core_slots[c], NT, off, keys, values, lens_i,
                          ge_all, W1T, W2T, WLT, b2bc, blbc, id_a)
        for c in range(N_CORES)
    ]
    res = run_bass_kernel_spmd(nc, in_maps, list(range(N_CORES)), trace=TRACE)
    global LAST_EXEC_NS
    LAST_EXEC_NS = res.exec_time_ns

    preds = np.zeros((B, T_DEC, VOCAB), np.float32)
    for c in range(N_CORES):
        out = res.results[c]["OUT"]  # (n_blk, 128, unroll, NVT, B_LOC)
        flat = np.asarray(out, np.float32).transpose(4, 0, 2, 3, 1) \
            .reshape(B_LOC, T_DEC, NVT * 128)
        for j in range(B_LOC):
            preds[core_slots[c][j]] = flat[j, :, :VOCAB]
    return preds
